# revision 30
# baseline (speedup 1.0000x reference)
"""Trainium2 Bass kernel for an R-GCN-style GCN layer (basis decomposition).

Reference computation (per relation r, with W_r = sum_b coeff[r,b] * basis[b]):
    out = sum_r segment_sum(inp[src_r] * val_r, dst_r) @ W_r + sum_r bias[r]

Algebraic restructure (4 basis accumulators instead of 16 relation matmuls):
    out[d] = sum_b G_b[d] @ basis[b] + bias_sum
    G_b[d] = sum_{edges e: dst_e = d} (coeff[r_e, b] * val_e) * inp[src_e]

Distribution: output nodes are sharded 8 ways (12500 rows/core); every core
holds the full gather table in its own HBM, so there is no cross-core
communication at all.

Per-core static structure (all shapes identical across cores; only data
differs, as SPMD requires):
  - 100 blocks of 128 dst nodes (98 real), grouped into 25 superblocks (SB)
    of 4 blocks.
  - Edges bucketed by (block, src-segment, group-of-32-dst-nodes). Src is
    split into 4 segments of 25000 so gather indices fit dma_gather's int16.
    Each bucket holds its first 128 edges in its own K=128 chunk; bucket
    overflow (data max 84 per block+segment) goes to one shared K=128 tail
    chunk per (block, segment) whose mask spans the whole block bank.
  - Per (SB, segment): ONE dma_gather of 2560 bf16 rows from a composite
    table with a zero row per segment (padding slots gather zeros). The 4
    segment gathers of an SB round-robin the 4 SWDGE queues so descriptor
    generation runs on all four Q7 core pairs concurrently (dma_gather
    only uses the Q7 pair cpu_id//2 == queue_num).
  - Per chunk: one fused DVE op builds the bf16 mask
      M[e, q*128 + n*4 + bb] = (dst_local[e] == n) * coeff[r_e, bb] * val_e
    (tail chunks compare dst%128 against a block-wide iota instead). One
    PE matmul accumulates gT[f, (q, n, bb)] += X_chunk.T @ M into the
    block's PSUM bank (fp32 accumulation).
  - Per block: 4 bf16 basis matmuls outT[fout, (q, n)] += basis_b.T @ gT_b,
    bias fused into the fp32 PSUM->SBUF copy on the scalar engine.

Output is produced transposed per block ([fout, node]) and reassembled on host.
"""
import os
import sys

for _p in ("/opt/trn_rl_repo", "/root/.axon_site/_ro/trn_rl_repo"):
    if os.path.isdir(_p) and _p not in sys.path:
        sys.path.insert(0, _p)

import numpy as np
import ml_dtypes

BF16NP = ml_dtypes.bfloat16

import concourse.bass as bass
import concourse.tile as tile
from concourse import bacc, mybir
from concourse.bass_utils import run_bass_kernel_spmd

# ---------------- problem constants (hardcoded from spec) ----------------
NN = 100000          # nodes
F = 128              # feature dim (in == out)
NB = 4               # bases
NREL = 16            # relations
NCORES = 8
NS = NN // NCORES    # dst nodes per core (12500)

GROUP = 32           # dst nodes per group
GPB = 4              # groups per block
BLOCK = GROUP * GPB  # 128 dst nodes per block
NBLK = 100           # padded block count (98 real)
BPS = 4              # blocks per superblock
NSB = NBLK // BPS    # 25 superblocks

NSEG = 4             # src segments
SEG = 25000          # src rows per segment
TBL_ROWS = NN + NSEG # composite table: one zero row per segment

BUCKETS = BPS * GPB  # 16 buckets per (SB, segment)
TAILCAP = 128        # shared overflow slots per (block, segment); data max 84
CS = BUCKETS + BPS   # 20 X columns per (SB, segment): 16 singles + 4 tails
SEG_IDX = CS * 128   # 2560 gather rows per (SB, segment)
COLS = NSEG * CS     # 80 X columns per SB

# meta layout per SB (bf16): [val: COLS][ldst: COLS][coef: 4*COLS]
META_COLS = 6 * COLS           # 480
IDX_COLS = NSEG * (SEG_IDX // 16)  # 640 int16 cols per SB

F32 = mybir.dt.float32
BF16 = mybir.dt.bfloat16
I16 = mybir.dt.int16

_compiled = {}


def _build_program():
    nc = bacc.Bacc(
        "TRN2",
        target_bir_lowering=False,
        debug=False,
        enable_asserts=False,
        num_devices=NCORES,
        num_swdge_queues=4,
    )

    tbl = nc.dram_tensor("tbl", [TBL_ROWS, F], BF16, kind="ExternalInput")
    basisw = nc.dram_tensor("basisw", [NB, F, F], BF16, kind="ExternalInput")
    biasw = nc.dram_tensor("biasw", [F, 1], F32, kind="ExternalInput")
    # iota with (n, bb) column order: col n*NB+bb holds n
    iota = nc.dram_tensor("iota", [128, NB * GROUP], BF16, kind="ExternalInput")
    # tail-mask iota over a whole block bank: col (qn, bb) holds qn (0..127)
    iota4 = nc.dram_tensor("iota4", [128, BLOCK * NB], BF16, kind="ExternalInput")
    eidx = nc.dram_tensor("eidx", [128, NSB * IDX_COLS], I16, kind="ExternalInput")
    meta = nc.dram_tensor("meta", [128, NSB * META_COLS], BF16, kind="ExternalInput")
    outT = nc.dram_tensor("outT", [NBLK, F, BLOCK], F32, kind="ExternalOutput")

    with tile.TileContext(nc) as tc:
        with (
            tc.tile_pool(name="const", bufs=1) as const,
            tc.tile_pool(name="xg", bufs=2) as xg,
            tc.tile_pool(name="idxp", bufs=2) as idxp,
            tc.tile_pool(name="metap", bufs=2) as metap,
            tc.tile_pool(name="w4p", bufs=2) as w4p,
            tc.tile_pool(name="msk", bufs=12) as mskp,
            tc.tile_pool(name="gt", bufs=4) as gtp,
            tc.tile_pool(name="ot", bufs=3) as otp,
            tc.tile_pool(name="psg", bufs=6, space="PSUM") as psg,
            tc.tile_pool(name="pso", bufs=2, space="PSUM") as pso,
        ):
            # ---- constants
            iota_t = const.tile([128, NB * GROUP], BF16)
            nc.sync.dma_start(out=iota_t[:], in_=iota[:, :])
            iota4_t = const.tile([128, BLOCK * NB], BF16)
            nc.sync.dma_start(out=iota4_t[:], in_=iota4[:, :])
            basis_t = const.tile([F, NB * F], BF16)
            for b in range(NB):
                nc.sync.dma_start(
                    out=basis_t[:, b * F : (b + 1) * F], in_=basisw[b, :, :]
                )
            # biasw is the host-precomputed column sum_r bias[r, :] ([F, 1])
            bias_col = const.tile([F, 1], F32)
            nc.sync.dma_start(out=bias_col[:], in_=biasw[:, :])

            for sb in range(NSB):
                idx_t = idxp.tile([128, IDX_COLS], I16)
                nc.sync.dma_start(
                    out=idx_t[:], in_=eidx[:, sb * IDX_COLS : (sb + 1) * IDX_COLS]
                )
                meta_t = metap.tile([128, META_COLS], BF16)
                nc.sync.dma_start(
                    out=meta_t[:], in_=meta[:, sb * META_COLS : (sb + 1) * META_COLS]
                )
                val_s = meta_t[:, 0:COLS]
                ldst_s = meta_t[:, COLS : 2 * COLS]
                coef_s = meta_t[:, 2 * COLS : META_COLS]

                # ---- gather: one dma_gather per src segment, queues round-robin
                x_t = xg.tile([128, COLS, F], BF16, tag="x")
                for s in range(NSEG):
                    nc.gpsimd.dma_gather(
                        out_ap=x_t[:, s * CS : (s + 1) * CS, :],
                        in_ap=tbl[s * (SEG + 1) :, :],
                        idxs_ap=idx_t[
                            :, s * (SEG_IDX // 16) : (s + 1) * (SEG_IDX // 16)
                        ],
                        num_idxs=SEG_IDX,
                        num_idxs_reg=SEG_IDX,
                        elem_size=F,
                        single_packet=False,
                        queue_num=s,
                    )

                # w4[e, col, bb] = val * coeff[r_e, bb]
                w4_t = w4p.tile([128, COLS * NB], BF16)
                nc.vector.tensor_mul(
                    w4_t[:].rearrange("p (c b) -> p c b", b=NB),
                    val_s[:, :, None].to_broadcast([128, COLS, NB]),
                    coef_s.rearrange("p (c b) -> p c b", b=NB),
                )
                w4_v = w4_t[:].rearrange("p (c b) -> p c b", b=NB)

                gt_ps = [
                    psg.tile([F, GPB * NB * GROUP], F32, tag="g", name=f"gt{b}")
                    for b in range(BPS)
                ]

                # region (b, q) chunk order: s-major; first chunk at s=0 is the
                # K=128 chunk, last at s=3 is the K=64 half.
                for s in range(NSEG):
                    for cis in range(CS):
                        col = s * CS + cis
                        # start=True arms a pending-zero for the WHOLE 2KB
                        # bank on trn2, so it must be issued exactly once per
                        # block bank (first matmul), never per q-region.
                        if cis < BUCKETS:
                            m_t = mskp.tile([128, NB * GROUP], BF16, tag="m")
                            nc.vector.scalar_tensor_tensor(
                                out=m_t[:].rearrange("p (n b) -> p n b", b=NB),
                                in0=iota_t[:].rearrange("p (n b) -> p n b", b=NB),
                                scalar=ldst_s[:, col : col + 1],
                                in1=w4_v[:, col : col + 1, :].to_broadcast(
                                    [128, GROUP, NB]
                                ),
                                op0=mybir.AluOpType.is_equal,
                                op1=mybir.AluOpType.mult,
                            )
                            bq = cis
                            b, q = bq // GPB, bq % GPB
                            nc.tensor.matmul(
                                gt_ps[b][:, q * 128 : (q + 1) * 128],
                                lhsT=x_t[:, col, :],
                                rhs=m_t[:],
                                start=(s == 0 and q == 0),
                                stop=False,
                                skip_group_check=True,
                            )
                        else:
                            # block tail: shared overflow slots for all 16
                            # buckets of block b in this segment; mask spans
                            # the whole block bank (ldst holds qn = dst%128).
                            b = cis - BUCKETS
                            m4_t = mskp.tile([128, BLOCK * NB], BF16, tag="m4")
                            nc.vector.scalar_tensor_tensor(
                                out=m4_t[:].rearrange("p (qn b) -> p qn b", b=NB),
                                in0=iota4_t[:].rearrange("p (qn b) -> p qn b", b=NB),
                                scalar=ldst_s[:, col : col + 1],
                                in1=w4_v[:, col : col + 1, :].to_broadcast(
                                    [128, BLOCK, NB]
                                ),
                                op0=mybir.AluOpType.is_equal,
                                op1=mybir.AluOpType.mult,
                            )
                            nc.tensor.matmul(
                                gt_ps[b][:, :],
                                lhsT=x_t[:, col, :],
                                rhs=m4_t[:],
                                start=False,
                                stop=(s == NSEG - 1),
                                skip_group_check=True,
                            )

                # ---- per block: basis application + bias + store
                for b in range(BPS):
                    j = sb * BPS + b
                    gt_sb = gtp.tile([F, GPB * NB * GROUP], BF16)
                    nc.scalar.copy(gt_sb[:], gt_ps[b][:])
                    ot_ps = pso.tile([F, BLOCK], F32)
                    gt_v = gt_sb[:].rearrange(
                        "p (q n b) -> p q n b", q=GPB, b=NB
                    )
                    for bb in range(NB):
                        nc.tensor.matmul(
                            ot_ps[:].rearrange("p (q n) -> p q n", q=GPB),
                            lhsT=basis_t[:, bb * F : (bb + 1) * F],
                            rhs=gt_v[:, :, :, bb],
                            start=(bb == 0),
                            stop=(bb == NB - 1),
                        )
                    ot_sb = otp.tile([F, BLOCK], F32)
                    nc.scalar.activation(
                        ot_sb[:],
                        ot_ps[:],
                        mybir.ActivationFunctionType.Identity,
                        bias=bias_col[:],
                    )
                    nc.sync.dma_start(out=outT[j, :, :], in_=ot_sb[:])

    nc.compile()
    return nc


def _preprocess(basis_coeff, edge_val, edge_src, edge_dst):
    """Pack edges into the static (SB, segment, bucket, chunk) structure.
    Returns per-core (eidx [128, NSB*IDX_COLS] int16,
    meta [128, NSB*META_COLS] bf16)."""
    src = np.ascontiguousarray(edge_src).ravel()
    dst = np.ascontiguousarray(edge_dst).ravel()
    val = np.ascontiguousarray(edge_val).ravel().astype(np.float32)
    rel = np.repeat(np.arange(NREL, dtype=np.int32), edge_src.shape[1])
    coeff = np.asarray(basis_coeff, dtype=np.float32)  # [NREL, NB]

    core = dst // NS
    per_core = []
    n_grp = NBLK * GPB  # 400 padded group slots (391 real)
    for c in range(NCORES):
        msel = core == c
        s_ = src[msel]
        dl = dst[msel] - c * NS
        v = val[msel]
        r = rel[msel]

        g = dl // GROUP                  # group 0..390
        w = (dl % GROUP).astype(np.float32)
        seg = s_ // SEG                  # 0..3
        lidx = (s_ % SEG + 1).astype(np.int16)  # 1..25000 (0 = zero row)

        bucket = g.astype(np.int64) * NSEG + seg
        order = np.argsort(bucket, kind="stable")
        s_, dl, v, r, g, w, seg, lidx, bucket = (
            a[order] for a in (s_, dl, v, r, g, w, seg, lidx, bucket)
        )
        cnt = np.bincount(bucket, minlength=n_grp * NSEG)
        starts = np.zeros(n_grp * NSEG + 1, dtype=np.int64)
        np.cumsum(cnt, out=starts[1:])
        pos = np.arange(len(s_)) - starts[bucket]

        # static slot map: (block j, q, seg, pos) -> (SB, X column, partition)
        j = g // GPB
        q = g % GPB
        sbi = j // BPS
        bis = (j % BPS) * GPB + q        # bucket index within (SB, seg), 0..15
        bib = j % BPS                    # block index within SB, 0..3
        in128 = pos < 128

        # bucket overflow (pos >= 128) goes to the per-(block, seg) tail
        # chunk; tail slot position = running count within (block, seg).
        tsel = ~in128
        tkey = (j.astype(np.int64) * NSEG + seg)[tsel]
        torder = np.argsort(tkey, kind="stable")
        tcnt = np.bincount(tkey, minlength=NBLK * NSEG)
        assert tcnt.max() <= TAILCAP, (
            f"tail capacity exceeded: {tcnt.max()} > {TAILCAP}"
        )
        tstarts = np.zeros(NBLK * NSEG + 1, dtype=np.int64)
        np.cumsum(tcnt, out=tstarts[1:])
        tpos_sorted = np.arange(tsel.sum()) - tstarts[tkey[torder]]
        tpos = np.empty(tsel.sum(), dtype=np.int64)
        tpos[torder] = tpos_sorted

        cis = np.where(in128, bis, BUCKETS + bib)
        part = pos.copy()
        part[tsel] = tpos
        # singles compare dst%32; tails compare dst%128 (position in block)
        w = np.where(in128, w, (dl % BLOCK).astype(np.float32))
        col = seg * CS + cis             # X column within SB, 0..79

        # gather position within (SB, seg): i = cis*128 + part
        gpos = cis * 128 + part

        # ---- index array: per (SB, seg) wrapped int16 [16, 192] tiled to 128
        idx_flat = np.zeros((NSB, NSEG, SEG_IDX), dtype=np.int16)
        idx_flat[sbi, seg, gpos] = lidx
        # wrap: position i = s16*16 + p16 -> [16, SEG_IDX//16]
        wrapped = idx_flat.reshape(NSB, NSEG, SEG_IDX // 16, 16).transpose(0, 1, 3, 2)
        # [NSB, NSEG, 16, 192] -> tile 16-partition pattern to 128 partitions
        wrapped = np.broadcast_to(
            wrapped[:, :, None, :, :], (NSB, NSEG, 8, 16, SEG_IDX // 16)
        ).reshape(NSB, NSEG, 128, SEG_IDX // 16)
        eidx_c = np.ascontiguousarray(
            wrapped.transpose(2, 0, 1, 3).reshape(128, NSB * IDX_COLS)
        )

        # ---- meta arrays [NSB, 128, META_COLS]
        mval = np.zeros((NSB, 128, COLS), dtype=np.float32)
        mldst = np.zeros((NSB, 128, COLS), dtype=np.float32)
        mcoef = np.zeros((NSB, 128, COLS, NB), dtype=np.float32)
        mval[sbi, part, col] = v
        mldst[sbi, part, col] = w
        mcoef[sbi, part, col] = coeff[r]
        meta_c = np.concatenate(
            [mval, mldst, mcoef.reshape(NSB, 128, COLS * NB)], axis=2
        )
        meta_c = np.ascontiguousarray(
            meta_c.transpose(1, 0, 2).reshape(128, NSB * META_COLS)
        ).astype(BF16NP)
        per_core.append((eidx_c, meta_c))
    return per_core


def _build_iota():
    # col (n, bb) -> n
    pat = np.repeat(np.arange(GROUP, dtype=np.float32), NB)
    io = np.ascontiguousarray(pat[None, :].repeat(128, 0))
    return io.astype(BF16NP)


def _build_iota4():
    # col (qn, bb) -> qn
    pat = np.repeat(np.arange(BLOCK, dtype=np.float32), NB)
    io4 = np.ascontiguousarray(pat[None, :].repeat(128, 0))
    return io4.astype(BF16NP)


def _build_table(inp):
    tbl = np.zeros((TBL_ROWS, F), dtype=np.float32)
    for s in range(NSEG):
        tbl[s * (SEG + 1) + 1 : (s + 1) * (SEG + 1)] = inp[s * SEG : (s + 1) * SEG]
    return tbl.astype(BF16NP)


def kernel(inp, basis_weights, basis_coeff, bias, edge_val, edge_src, edge_dst):
    inp = np.ascontiguousarray(np.asarray(inp, dtype=np.float32))
    basis_weights = np.ascontiguousarray(np.asarray(basis_weights, dtype=np.float32))
    basis_coeff = np.asarray(basis_coeff, dtype=np.float32)
    bias = np.ascontiguousarray(np.asarray(bias, dtype=np.float32))

    if "nc" not in _compiled:
        _compiled["nc"] = _build_program()
    nc = _compiled["nc"]

    per_core = _preprocess(basis_coeff, edge_val, edge_src, edge_dst)
    tbl = _build_table(inp)
    iota_np = _build_iota()
    iota4_np = _build_iota4()
    basisw_np = basis_weights.astype(BF16NP)

    in_maps = []
    for c in range(NCORES):
        eidx_c, meta_c = per_core[c]
        in_maps.append(
            {
                "tbl": tbl,
                "basisw": basisw_np,
                "biasw": np.ascontiguousarray(bias.sum(axis=0)[:, None]),
                "iota": iota_np,
                "iota4": iota4_np,
                "eidx": eidx_c,
                "meta": meta_c,
            }
        )

    res = run_bass_kernel_spmd(nc, in_maps, list(range(NCORES)))
    _compiled["last_results"] = res

    out = np.empty((NN, F), dtype=np.float32)
    for c in range(NCORES):
        oT = res.results[c]["outT"]  # [NBLK, F, BLOCK]
        rows = oT.transpose(0, 2, 1).reshape(NBLK * BLOCK, F)[:NS]
        out[c * NS : (c + 1) * NS] = rows
    return out
